# revision 1
# baseline (speedup 1.0000x reference)
"""FCOS detection head on 8 Trainium2 NeuronCores.

Sharding: 4 independent streams (image x {cls,reg} tower-branch), each split
across a pair of cores by rows of the two big FPN levels (L0, L1); the three
small levels are replicated within the pair. GroupNorm statistics for the
split levels are combined with tiny pairwise AllReduces (one per GN layer
per round). Convs run as 9-shifted-offset matmuls in float32r (FP22) with
fp32 PSUM accumulation; small tiles fall back to true fp32 at equal speed.
"""
import sys
sys.path.insert(0, '/opt/trn_rl_repo')

import numpy as np
import concourse.bass as bass
import concourse.bacc as bacc
import concourse.tile as tile
from concourse import mybir
from concourse.bass_utils import run_bass_kernel_spmd

F32 = mybir.dt.float32
F32R = mybir.dt.float32r
ALU = mybir.AluOpType
AF = mybir.ActivationFunctionType

N_CORES = 8
RG = [[0, 1], [2, 3], [4, 5], [6, 7]]
CFPN = 256
NCK = 2          # 256 channels = 2 partition chunks of 128
HEAD_CH = 85     # 80 cls + 4 box + 1 ctr
GN_EPS = 1e-5
N_BATCH = 2


class Lv:
    def __init__(self, idx, H, W, R, T, split, g_conv, g_stats, g_head):
        self.idx, self.H, self.W, self.R, self.T = idx, H, W, R, T
        self.split = split
        self.g_conv, self.g_stats, self.g_head = g_conv, g_stats, g_head
        self.region = R + 2 * T           # rows held on-core (owned + borders)
        self.Wp = W + 2
        if (self.region * self.Wp) % 2 == 1 and self.region * self.Wp >= 256:
            self.Wp += 1                  # keep f32r tiles even-sized
        self.bufrows = self.region + 4    # + 2 guard rows top, 2 bottom
        self.ndiv = 16 * (2 if split else 1)  # channels/group x cores averaged


_SPECS = [
    (100, 152, 50, 4, True, 3, 2, 3),
    (50, 76, 25, 4, True, 6, 5, 6),
    (25, 38, 25, 1, False, 12, 5, 12),
    (13, 19, 13, 1, False, 15, 13, 13),
    (7, 10, 7, 1, False, 9, 7, 7),
]
LEVELS = [Lv(i, *s) for i, s in enumerate(_SPECS)]

# packed input-feature blob: per (level, chunk) blocks of [128, region*W]
XIN_OFF = {}
_off = 0
for lv in LEVELS:
    for ck in range(NCK):
        XIN_OFF[(lv.idx, ck)] = _off
        _off += lv.region * lv.W
XIN_COLS = _off

OUT_BASE = {}
_ob = 0
for lv in LEVELS:
    OUT_BASE[lv.idx] = _ob
    _ob += lv.R * lv.W
OUT_PX = _ob

# split level first per round; its replicated companions fill the
# AllReduce wait with local conv work
ROUNDS = [[0, 3, 4], [1, 2]]


def _row_tiles(nrows, g):
    """Balanced [(r0, cnt)] covering nrows with ceil(nrows/g) tiles.

    Tile sizes differ by at most 1, keeping every tile's free size even
    (all Wp are even) and >= 256 px so conv matmuls stay on the f32r
    fast path instead of the 4x-slower fp32 fallback."""
    ntiles = -(-nrows // g)
    base, rem = divmod(nrows, ntiles)
    out = []
    r = 0
    for i in range(ntiles):
        cnt = base + (1 if i < rem else 0)
        out.append((r, cnt))
        r += cnt
    return out


def build_program():
    nc = bacc.Bacc("TRN2", target_bir_lowering=False)

    xin = nc.dram_tensor("xin", [128, XIN_COLS], F32R, kind="ExternalInput")
    wt = nc.dram_tensor("wt", [3, 128, NCK, 9, CFPN], F32R, kind="ExternalInput")
    wh = nc.dram_tensor("wh", [128, NCK, 9, HEAD_CH], F32R, kind="ExternalInput")
    pc = nc.dram_tensor("pc", [128, 3, 3, NCK], F32, kind="ExternalInput")
    mk = nc.dram_tensor("mk", [128, 5, 2], F32, kind="ExternalInput")
    hp = nc.dram_tensor("hp", [HEAD_CH, 2], F32, kind="ExternalInput")
    g8 = nc.dram_tensor("g8", [128, 8], F32, kind="ExternalInput")
    idm = nc.dram_tensor("idm", [128, 128], F32, kind="ExternalInput")
    out = nc.dram_tensor("out", [OUT_PX, HEAD_CH], F32, kind="ExternalOutput")

    with tile.TileContext(nc) as tc:
        _emit(nc, tc, xin, wt, wh, pc, mk, hp, g8, idm, out)
    return nc


def _emit(nc, tc, xin, wt, wh, pc, mk, hp, g8, idm, out):
    from contextlib import ExitStack
    ctx = ExitStack()
    persist = ctx.enter_context(tc.tile_pool(name="persist", bufs=1))
    bufs = ctx.enter_context(tc.tile_pool(name="bufs", bufs=1))
    wpool = ctx.enter_context(tc.tile_pool(name="wpool", bufs=1))
    small = ctx.enter_context(tc.tile_pool(name="small", bufs=6))
    bnpool = ctx.enter_context(tc.tile_pool(name="bnpool", bufs=2))
    hstg = ctx.enter_context(tc.tile_pool(name="hstg", bufs=2))
    ostg = ctx.enter_context(tc.tile_pool(name="ostg", bufs=2))
    psA = ctx.enter_context(tc.tile_pool(name="psA", bufs=4, space="PSUM"))
    psT = ctx.enter_context(tc.tile_pool(name="psT", bufs=2, space="PSUM"))
    psS = ctx.enter_context(tc.tile_pool(name="psS", bufs=2, space="PSUM"))
    dpool = ctx.enter_context(tc.tile_pool(name="dpool", bufs=1, space="DRAM"))

    # ---- persistent small data (DMAs emitted after round-1 input loads
    # so they don't delay the first conv tiles in the DMA queues)
    g8t = persist.tile([128, 8], F32, name="g8t")
    idt = persist.tile([128, 128], F32, name="idt")
    pct = persist.tile([128, 3, 3, NCK], F32, name="pct")
    mkt = persist.tile([128, 5, 2], F32, name="mkt")
    hpt = persist.tile([HEAD_CH, 2], F32, name="hpt")
    epst = persist.tile([128, 1], F32, name="epst")
    wht = persist.tile([128, NCK, 9, HEAD_CH], F32R, name="wht")

    def emit_persist_loads():
        nc.sync.dma_start(out=g8t, in_=g8[:, :])
        nc.sync.dma_start(out=idt, in_=idm[:, :])
        nc.sync.dma_start(out=pct, in_=pc[:, :, :, :])
        nc.sync.dma_start(out=mkt, in_=mk[:, :, :])
        nc.sync.dma_start(out=hpt, in_=hp[:, :])
        nc.vector.memset(epst, GN_EPS)
        nc.sync.dma_start(out=wht, in_=wh[:, :, :, :])

    def conv_stats_level(li, j, A, B, wsb, ccin, locj, loc_off):
        """3x3 conv (no bias) from A into B, with per-tile partial sums of
        y and y^2 over the owned rows interleaved with the evictions, so
        the GN stats chain is nearly done when the last matmul retires."""
        lv = LEVELS[li]
        Wp, region, T, R, W = lv.Wp, lv.region, lv.T, lv.R, lv.W
        Afl = {ck: A[li][ck].rearrange("p r w -> p (r w)") for ck in range(NCK)}
        Bfl = {ck: B[li][ck].rearrange("p r w -> p (r w)") for ck in range(NCK)}
        # rows whose conv output is still needed at tower depth j
        rlo = max(0, T - 3 + j)
        rhi = min(region, T + R + 3 - j)
        tiles = [(rlo + r0, g) for (r0, g) in _row_tiles(rhi - rlo, lv.g_conv)]
        nstat = sum(1 for (r0, g) in tiles if max(r0, T) < min(r0 + g, T + R))
        pa = {ck: bnpool.tile([128, nstat], F32, name="pa", tag=f"pa{ck}")
              for ck in range(NCK)}
        pb = {ck: bnpool.tile([128, nstat], F32, name="pb", tag=f"pb{ck}")
              for ck in range(NCK)}
        tix = {0: 0, 1: 0}
        for (r0, g) in tiles:
            n = g * Wp
            use_r = n >= 256 and n % 2 == 0
            base = (2 + r0) * Wp
            for oc in range(NCK):
                ps = psA.tile([128, n], F32, name="ps_conv", tag="psa")
                k = 0
                for ick in range(NCK):
                    rhs_full = Afl[ick]
                    for dy in range(3):
                        for dx in range(3):
                            sh = (dy - 1) * Wp + (dx - 1)
                            rhs = rhs_full[:, base + sh: base + sh + n]
                            lhsT = wsb[:, ick, dy * 3 + dx,
                                       oc * 128:(oc + 1) * 128]
                            if not use_r:
                                rhs = rhs.bitcast(F32)
                                lhsT = lhsT.bitcast(F32)
                            nc.tensor.matmul(ps, lhsT, rhs,
                                             start=(k == 0), stop=(k == 17))
                            k += 1
                ps3 = ps.rearrange("p (r w) -> p r w", w=Wp)
                lo, hi = max(r0, T), min(r0 + g, T + R)
                if lo == r0 and hi == r0 + g:
                    # fully-owned tile: evict real cols and accumulate
                    # sum(y) in the same DVE op, then fused square+sum
                    t = tix[oc]
                    tix[oc] += 1
                    bsl = B[li][oc][:, r0:r0 + g, 1:1 + W]
                    nc.vector.tensor_scalar(
                        out=bsl, in0=ps3[:, :, 1:1 + W], scalar1=1.0,
                        scalar2=0.0, op0=ALU.mult, op1=ALU.add,
                        accum_out=pa[oc][:, t:t + 1])
                    scr = bnpool.tile([128, g, W], F32,
                                      name="sqscr", tag="sqscr")
                    nc.vector.scalar_tensor_tensor(
                        out=scr, in0=bsl, scalar=1.0, in1=bsl,
                        op0=ALU.mult, op1=ALU.mult,
                        accum_out=pb[oc][:, t:t + 1])
                else:
                    nc.vector.tensor_copy(
                        out=Bfl[oc][:, r0 * Wp: r0 * Wp + n], in_=ps)
                    if lo < hi:
                        t = tix[oc]
                        tix[oc] += 1
                        bsl = B[li][oc][:, lo:hi, 1:1 + W]
                        nc.vector.tensor_reduce(
                            out=pa[oc][:, t:t + 1], in_=bsl,
                            axis=mybir.AxisListType.XY, op=ALU.add)
                        scr = bnpool.tile([128, hi - lo, W], F32,
                                          name="sqscr", tag="sqscr")
                        nc.vector.tensor_mul(out=scr, in0=bsl, in1=bsl)
                        nc.vector.tensor_reduce(
                            out=pb[oc][:, t:t + 1], in_=scr,
                            axis=mybir.AxisListType.XY, op=ALU.add)
        # fold partials -> per-channel (mean_y+b, E[(y+b)^2]) -> group sums
        ninv = 1.0 / float(R * W)
        for ck in range(NCK):
            cb = pct[:, 0, j, ck:ck + 1]
            sa = small.tile([128, 1], F32, name="sa", tag="sa")
            nc.vector.tensor_reduce(out=sa, in_=pa[ck],
                                    axis=mybir.AxisListType.X, op=ALU.add)
            sb = small.tile([128, 1], F32, name="sb", tag="sb")
            nc.vector.tensor_reduce(out=sb, in_=pb[ck],
                                    axis=mybir.AxisListType.X, op=ALU.add)
            t12 = small.tile([128, 2], F32, name="t12", tag="t12")
            # t12[0] = Sa/n + b
            nc.vector.scalar_tensor_tensor(
                out=t12[:, 0:1], in0=sa, scalar=ninv, in1=cb,
                op0=ALU.mult, op1=ALU.add)
            # t12[1] = Sb/n + b*(2*Sa/n + b) = E[(y+b)^2]
            u = small.tile([128, 1], F32, name="u", tag="u")
            nc.vector.scalar_tensor_tensor(
                out=u, in0=sa, scalar=2.0 * ninv, in1=cb,
                op0=ALU.mult, op1=ALU.add)
            w1 = small.tile([128, 1], F32, name="w1", tag="w1")
            nc.vector.tensor_mul(out=w1, in0=u, in1=cb)
            nc.vector.scalar_tensor_tensor(
                out=t12[:, 1:2], in0=sb, scalar=ninv, in1=w1,
                op0=ALU.mult, op1=ALU.add)
            stps = psS.tile([8, 2], F32, name="stps", tag="stps")
            nc.tensor.matmul(stps, g8t, t12, start=True, stop=True)
            stsb = small.tile([8, 2], F32, name="stsb", tag="stsb")
            nc.vector.tensor_copy(out=stsb, in_=stps)
            if lv.split:
                dst = ccin[ck * 8:(ck + 1) * 8, :]
            else:
                ro = loc_off[li] + ck * 8
                dst = locj[ro:ro + 8, :]
            nc.gpsimd.dma_start(out=dst, in_=stsb)

    def gn_apply(lvset, j, A, B, ccout, locj, loc_off):
      # per level: both chunks' alpha/beta chains first (one Sqrt table
      # load), then both relu applications (one Relu table load) — and the
      # split level's whole pipeline stays ahead of the replicated ones
      for li in lvset:
        if True:
            coeffs = []
            lv = LEVELS[li]
            for ck in range(NCK):
                if lv.split:
                    src = ccout[ck * 8:(ck + 1) * 8, :]
                else:
                    ro = loc_off[li] + ck * 8
                    src = locj[ro:ro + 8, :]
                bsrc = bass.AP(tensor=src.tensor, offset=src.offset,
                               ap=[[2, 8], [0, 16], [1, 2]])
                mv128 = small.tile([128, 2], F32, name="mv128", tag="mv128")
                nc.gpsimd.dma_start(out=mv128, in_=bsrc)
                me2 = small.tile([128, 2], F32, name="me2", tag="me2")
                nc.vector.tensor_scalar_mul(out=me2, in0=mv128,
                                            scalar1=1.0 / lv.ndiv)
                vr = small.tile([128, 1], F32, name="vr", tag="vr")
                nc.vector.scalar_tensor_tensor(
                    out=vr, in0=me2[:, 0:1], scalar=-1.0, in1=me2[:, 0:1],
                    op0=ALU.mult, op1=ALU.mult)   # -mean^2
                nc.vector.tensor_add(out=vr, in0=me2[:, 1:2], in1=vr)
                sd = small.tile([128, 1], F32, name="sd", tag="sd")
                nc.scalar.activation(out=sd, in_=vr, func=AF.Sqrt,
                                     bias=epst, scale=1.0)
                rstd = small.tile([128, 1], F32, name="rstd", tag="rstd")
                nc.vector.reciprocal(out=rstd, in_=sd)
                al = small.tile([128, 1], F32, name="al", tag="al")
                nc.vector.tensor_mul(out=al, in0=pct[:, 1, j, ck:ck + 1],
                                     in1=rstd)
                bt = small.tile([128, 1], F32, name="bt", tag="bt")
                nc.vector.tensor_tensor(out=bt, in0=pct[:, 0, j, ck:ck + 1],
                                        in1=me2[:, 0:1], op=ALU.subtract)
                be = small.tile([128, 1], F32, name="be", tag="be")
                nc.vector.scalar_tensor_tensor(
                    out=be, in0=bt, scalar=al, in1=pct[:, 2, j, ck:ck + 1],
                    op0=ALU.mult, op1=ALU.add)
                coeffs.append((li, ck, al, be))
        # phase 2: relu(al*y + be) in row-chunks for pipelining, writing
        # only the real W columns (pad columns stay zero from round init).
        # Border masks fire right after their own chunk so the next conv's
        # first tiles aren't gated on the whole level.
        for (li, ck, al, be) in coeffs:
            lv = LEVELS[li]
            region = lv.region
            T, R, W = lv.T, lv.R, lv.W
            q = max(1, R // 4)
            chunks = [(0, T)] + [(T + s, min(q, R - s)) for s in
                                 range(0, R, q)] + [(T + R, T)]
            mtop = mkt[:, li, 0:1]
            mbot = mkt[:, li, 1:2]
            for (c0, cn) in chunks:
                if cn <= 0:
                    continue
                nc.scalar.activation(
                    out=A[li][ck][:, 2 + c0:2 + c0 + cn, 1:1 + W],
                    in_=B[li][ck][:, c0:c0 + cn, 1:1 + W],
                    func=AF.Relu, bias=be, scale=al)
                if c0 == 0:
                    nc.vector.tensor_scalar_mul(
                        out=A[li][ck][:, 2:2 + T, 1:1 + W],
                        in0=A[li][ck][:, 2:2 + T, 1:1 + W], scalar1=mtop)
                if c0 == T + R:
                    nc.vector.tensor_scalar_mul(
                        out=A[li][ck][:, 2 + T + R:2 + region, 1:1 + W],
                        in0=A[li][ck][:, 2 + T + R:2 + region, 1:1 + W],
                        scalar1=mbot)

    def head_level(li, A):
        lv = LEVELS[li]
        Wp, T, R, W = lv.Wp, lv.T, lv.R, lv.W
        Afl = {ck: A[li][ck].rearrange("p r w -> p (r w)") for ck in range(NCK)}
        hb = hpt[:, 0:1]
        mrelu = hpt[:, 1:2]
        for (r0, g) in _row_tiles(R, lv.g_head):
            n = g * Wp
            use_r = n >= 256 and n % 2 == 0
            base = (2 + T + r0) * Wp
            ps = psA.tile([HEAD_CH, n], F32, name="ps_head", tag="psa")
            k = 0
            for ick in range(NCK):
                for dy in range(3):
                    for dx in range(3):
                        sh = (dy - 1) * Wp + (dx - 1)
                        rhs = Afl[ick][:, base + sh: base + sh + n]
                        lhsT = wht[:, ick, dy * 3 + dx, :]
                        if not use_r:
                            rhs = rhs.bitcast(F32)
                            lhsT = lhsT.bitcast(F32)
                        nc.tensor.matmul(ps, lhsT, rhs,
                                         start=(k == 0), stop=(k == 17))
                        k += 1
            hs = hstg.tile([HEAD_CH, g * W], F32, name="hs", tag="hs")
            ps3 = ps.rearrange("p (r w) -> p r w", w=Wp)
            hs3 = hs.rearrange("p (r w) -> p r w", w=W)
            nc.vector.tensor_scalar_add(out=hs3, in0=ps3[:, :, 1:1 + W],
                                        scalar1=hb)
            # selective relu: max(m*u, u); m=0 -> relu, m=1 -> identity
            nc.vector.scalar_tensor_tensor(
                out=hs, in0=hs, scalar=mrelu, in1=hs,
                op0=ALU.mult, op1=ALU.max)
            # transpose 128-px blocks and store [px, 85] rows
            px0 = OUT_BASE[li] + r0 * W
            npx = g * W
            for b0 in range(0, npx, 128):
                nb = min(128, npx - b0)
                tp = psT.tile([128, HEAD_CH], F32, name="tp", tag="tp")
                nc.tensor.transpose(tp[:nb, :], hs[:, b0:b0 + nb],
                                    idt[:HEAD_CH, :HEAD_CH])
                ot = ostg.tile([128, HEAD_CH], F32, name="ot", tag="ot")
                nc.vector.tensor_copy(out=ot[:nb, :], in_=tp[:nb, :])
                nc.sync.dma_start(out=out[px0 + b0:px0 + b0 + nb, :],
                                  in_=ot[:nb, :])

    # ================= rounds =================
    for ri, lvset in enumerate(ROUNDS):
        A, B = {}, {}
        wsb0 = wpool.tile([128, NCK, 9, CFPN], F32R, name="wsb", tag="wsb")
        for ick in range(NCK):
            for off in range(9):
                nc.sync.dma_start(out=wsb0[:, ick, off, :],
                                  in_=wt[0, :, ick, off, :])
        for li in lvset:
            lv = LEVELS[li]
            A[li] = {}
            B[li] = {}
            for ck in range(NCK):
                a = bufs.tile([128, lv.bufrows, lv.Wp], F32R,
                              name=f"A{li}c{ck}", tag=f"Ac{ck}" if li in (0, 1) else f"A{li}c{ck}")
                nc.vector.memset(a[:, 0:2, :].bitcast(F32), 0.0)
                nc.vector.memset(
                    a[:, 2 + lv.region:lv.bufrows, :].bitcast(F32), 0.0)
                nc.vector.memset(
                    a[:, 2:2 + lv.region, 0:1].bitcast(F32), 0.0)
                nc.vector.memset(
                    a[:, 2:2 + lv.region, 1 + lv.W:lv.Wp].bitcast(F32), 0.0)
                A[li][ck] = a
                b = bufs.tile([128, lv.region, lv.Wp], F32,
                              name=f"B{li}c{ck}", tag=f"Bc{ck}" if li in (0, 1) else f"B{li}c{ck}")
                B[li][ck] = b
            for (q0, qn) in _row_tiles(lv.region, max(4, lv.region // 4)):
                for ck in range(NCK):
                    o = XIN_OFF[(li, ck)] + q0 * lv.W
                    nc.sync.dma_start(
                        out=A[li][ck][:, 2 + q0:2 + q0 + qn, 1:1 + lv.W],
                        in_=xin[:, o:o + qn * lv.W]
                        .rearrange("p (r w) -> p r w", w=lv.W))

        if ri == 0:
            emit_persist_loads()
        has_split = any(LEVELS[li].split for li in lvset)
        repl_set = [li for li in lvset if not LEVELS[li].split]
        split_set = [li for li in lvset if LEVELS[li].split]

        loc_off = {li: pos * 16 for pos, li in enumerate(repl_set)}
        for j in range(3):
            if j == 0:
                wsb = wsb0
            else:
                wsb = wpool.tile([128, NCK, 9, CFPN], F32R, name=f"wsb",
                                 tag="wsb")
                for ick in range(NCK):
                    nc.sync.dma_start(out=wsb[:, ick, :, :],
                                      in_=wt[j, :, ick, :, :])
            ccin = ccout = locj = None
            if repl_set:
                locj = dpool.tile([16 * len(repl_set), 2], F32,
                                  name=f"loc_{ri}_{j}", tag=f"loc_{ri}_{j}")
            if has_split:
                ccin = dpool.tile([16, 2], F32, name=f"ccin_{ri}_{j}",
                                  tag=f"ccin_{ri}_{j}")
                ccout = dpool.tile([16, 2], F32, name=f"ccout_{ri}_{j}",
                                   tag=f"ccout_{ri}_{j}")
            # per-level conv+stats so each level's GN inputs are ready
            # as early as possible (replicated levels fill the cc wait)
            for li in lvset:
                conv_stats_level(li, j, A, B, wsb, ccin, locj, loc_off)
            if has_split:
                nc.gpsimd.collective_compute(
                    "AllReduce", ALU.add, ins=[ccin[:, :]],
                    outs=[ccout[:, :]], replica_groups=RG)
            gn_apply(split_set + repl_set, j, A, B, ccout, locj, loc_off)

        for li in lvset:
            head_level(li, A)

    ctx.close()


# ===================== host side =====================

_CACHE = {}
_last_results = None


def _pack_core(feats, tower_w, tower_b, gn_s, gn_b, head_w, head_b, head_m,
               img, half):
    """Build the per-core input dict (numpy) for one (img, branch, half)."""
    xin = np.zeros((128, XIN_COLS), np.float32)
    for lv in LEVELS:
        f = feats[lv.idx][img]  # [256, H, W]
        own0 = half * lv.R if lv.split else 0
        r_lo = own0 - lv.T
        for ck in range(NCK):
            blk = np.zeros((128, lv.region, lv.W), np.float32)
            for r in range(lv.region):
                gr = r_lo + r
                if 0 <= gr < lv.H:
                    blk[:, r, :] = f[ck * 128:(ck + 1) * 128, gr, :]
            o = XIN_OFF[(lv.idx, ck)]
            xin[:, o:o + lv.region * lv.W] = blk.reshape(128, -1)

    # tower weights [3,256out,256in,3,3] -> [3, 128ic, 2ick, 9, 256oc]
    w = np.transpose(tower_w, (0, 2, 3, 4, 1)).reshape(3, 2, 128, 9, 256)
    wtp = np.ascontiguousarray(np.transpose(w, (0, 2, 1, 3, 4)))

    # head weights [85, 256, 3, 3] -> [128ic, 2ick, 9, 85]
    hw = np.transpose(head_w, (1, 2, 3, 0)).reshape(2, 128, 9, HEAD_CH)
    whp = np.ascontiguousarray(np.transpose(hw, (1, 0, 2, 3)))

    # per-channel params [128, 3 param, 3 layer, 2 chunk]
    pcp = np.zeros((128, 3, 3, NCK), np.float32)
    for j in range(3):
        for ck in range(NCK):
            sl = slice(ck * 128, (ck + 1) * 128)
            pcp[:, 0, j, ck] = tower_b[j][sl]
            pcp[:, 1, j, ck] = gn_s[j][sl]
            pcp[:, 2, j, ck] = gn_b[j][sl]

    mkp = np.zeros((128, 5, 2), np.float32)
    for lv in LEVELS:
        if lv.split:
            mkp[:, lv.idx, 0] = 0.0 if half == 0 else 1.0
            mkp[:, lv.idx, 1] = 1.0 if half == 0 else 0.0

    hpp = np.zeros((HEAD_CH, 2), np.float32)
    hpp[:, 0] = head_b
    hpp[:, 1] = head_m

    g8p = np.zeros((128, 8), np.float32)
    for i in range(128):
        g8p[i, i // 16] = 1.0

    return {
        "xin": xin,
        "wt": wtp,
        "wh": whp,
        "pc": pcp,
        "mk": mkp,
        "hp": hpp,
        "g8": g8p,
        "idm": np.eye(128, dtype=np.float32),
    }


def kernel(feat0, feat1, feat2, feat3, feat4,
           cls_conv_w, cls_conv_b, cls_gn_s, cls_gn_b, cls_out_w, cls_out_b,
           reg_conv_w, reg_conv_b, reg_gn_s, reg_gn_b,
           box_w, box_b, ctr_w, ctr_b):
    global _last_results
    feats = [np.asarray(f, np.float32) for f in
             (feat0, feat1, feat2, feat3, feat4)]

    if "nc" not in _CACHE:
        _CACHE["nc"] = build_program()
        _CACHE["nc"].finalize()
    nc = _CACHE["nc"]

    # branch-specific packed weights
    allw = np.concatenate([np.asarray(cls_out_w, np.float32),
                           np.asarray(box_w, np.float32),
                           np.asarray(ctr_w, np.float32)], axis=0)  # [85,...]
    allb = np.concatenate([np.asarray(cls_out_b, np.float32),
                           np.asarray(box_b, np.float32),
                           np.asarray(ctr_b, np.float32)])
    w_cls = allw.copy(); w_cls[80:] = 0.0
    w_reg = allw.copy(); w_reg[:80] = 0.0
    b_cls = allb.copy(); b_cls[80:] = 0.0
    b_reg = allb.copy(); b_reg[:80] = 0.0
    m_cls = np.ones(HEAD_CH, np.float32)           # identity everywhere
    m_reg = np.ones(HEAD_CH, np.float32)
    m_reg[80:84] = 0.0                             # relu on box channels

    branch_args = {
        0: (np.asarray(cls_conv_w, np.float32), np.asarray(cls_conv_b, np.float32),
            np.asarray(cls_gn_s, np.float32), np.asarray(cls_gn_b, np.float32),
            w_cls, b_cls, m_cls),
        1: (np.asarray(reg_conv_w, np.float32), np.asarray(reg_conv_b, np.float32),
            np.asarray(reg_gn_s, np.float32), np.asarray(reg_gn_b, np.float32),
            w_reg, b_reg, m_reg),
    }

    in_maps = []
    for core in range(N_CORES):
        img = core // 4
        br = (core // 2) % 2
        half = core % 2
        tw, tb, gs, gb, hw, hb, hm = branch_args[br]
        in_maps.append(_pack_core(feats, tw, tb, gs, gb, hw, hb, hm,
                                  img, half))

    res = run_bass_kernel_spmd(nc, in_maps, core_ids=list(range(N_CORES)))
    _last_results = res

    fullout = np.zeros((N_BATCH, 20267, 85), np.float32)
    GBASE = {0: 0, 1: 15200, 2: 19000, 3: 19950, 4: 20197}
    for core in range(N_CORES):
        img = core // 4
        br = (core // 2) % 2
        half = core % 2
        ch = slice(0, 80) if br == 0 else slice(80, 85)
        o = res.results[core]["out"]
        for lv in LEVELS:
            n = lv.R * lv.W
            src = o[OUT_BASE[lv.idx]:OUT_BASE[lv.idx] + n, ch]
            if lv.split:
                d0 = GBASE[lv.idx] + half * n
                fullout[img, d0:d0 + n, ch] = src
            elif half == 0:
                fullout[img, GBASE[lv.idx]:GBASE[lv.idx] + n, ch] = src
    return fullout

